# revision 66
# baseline (speedup 1.0000x reference)
"""Trainium2 Bass kernel for AttnDecoderRNN single-step (N=64, T=800, H=512).

Strategy (8 NeuronCores, SPMD single program):
  - Weight GEMM phases are model-parallel: each core owns a 1/8 feature slice
    of every weight matrix; activations stay feature-major [feat, batch=64];
    AllGather (partition-axis concat) rebuilds full activations between GEMMs.
  - The large attention middle (e = tanh(attW + dec_proj), score, weighted
    sum over T with ragged length masking) is batch-parallel: rows are sorted
    by length and dealt round-robin to cores so every "row slot" s has a
    uniform compile-time chunk count T_s; per-core raggedness lives entirely
    in the (per-core) input data, keeping the program identical across cores.
  - dec_proj rows are routed to the core that owns each batch row with one
    AllToAll, then broadcast across partitions for the e-add.
Host side packs weight slices / biases / ragged-packed attW & enc buffers per
core, and reassembles + unpermutes outputs.
"""

import math
import sys

for _p in ("/opt/trn_rl_repo",):
    if _p not in sys.path:
        sys.path.insert(0, _p)

import ml_dtypes
import numpy as np

BF16NP = ml_dtypes.bfloat16

import concourse.bass as bass
import concourse.mybir as mybir
import concourse.tile as tile
from concourse import bacc
from concourse.bass_utils import run_bass_kernel_spmd
from concourse.masks import make_identity

F32 = mybir.dt.float32
BF16 = mybir.dt.bfloat16
AF = mybir.ActivationFunctionType
ALU = mybir.AluOpType

N, T, H, O, R = 64, 800, 512, 80, 2
H2 = 2 * H            # 1024
KO = R * O            # 160
P = 128               # partitions
NCORE = 8
B = N                 # batch (free dim in GEMM phases)
NSLOT = N // NCORE    # 8 row slots per core


# ---------------------------------------------------------------------------
# host-side weight packing
# ---------------------------------------------------------------------------
class Pack:
    """Concatenate [128, m] blocks along the free dim; identical key->offset
    mapping on every core (shapes are core-independent)."""

    def __init__(self):
        self.off = {}
        self.pos = 0
        self.order = []

    def add(self, key, m):
        assert key not in self.off
        self.off[key] = (self.pos, m)
        self.order.append(key)
        self.pos += m

    def build(self, blocks):
        out = np.zeros((P, self.pos), BF16NP)
        for key, arr in blocks.items():
            o, m = self.off[key]
            assert arr.shape == (P, m), (key, arr.shape, m)
            out[:, o : o + m] = arr.astype(BF16NP)
        return out


def _lhsT(w, rows, kchunk):
    """w[rows, 128*kchunk : 128*(kchunk+1)].T as [128, len(rows)] (K zero-pad)."""
    blk = w[rows, P * kchunk : P * (kchunk + 1)].T.astype(np.float32)
    if blk.shape[0] < P:
        blk = np.pad(blk, ((0, P - blk.shape[0]), (0, 0)))
    return np.ascontiguousarray(blk)


def _make_pack_layout(zeros_h):
    """zeros_h: initial hidden states are all-zero (guaranteed by the input
    spec); drops every Whh @ h0 GEMM and adds replicated Wih_att/W_ld for the
    collective-free dec_proj fast path."""
    pk1 = Pack()
    for m in range(8):          # prenet replicated: full output dims
        pk1.add(("pre1", m), P)
    for m in range(4):
        for k in range(8):
            pk1.add(("pre2", m, k), P)
    for g in range(3):
        for k in range(4):
            pk1.add(("att_ih", g, k), P)
    if not zeros_h:
        for g in range(3):
            for k in range(8):
                pk1.add(("att_hh", g, k), P)
    for h in range(2):
        pk1.add(("ldT", h), 512)
    # replicated fast-path weights in their own tensor so its DMA is staged
    pk1b = Pack()
    if zeros_h:
        for g in range(3):
            for m in range(8):
                for k in range(4):
                    pk1b.add(("ihf", g, m, k), P)
        for m in range(8):
            for k in range(8):
                pk1b.add(("ldf", m, k), P)

    pk3 = Pack()
    for k in range(16):
        pk3.add(("sc", k), P)
    for g in range(3):
        for k in range(16):
            pk3.add(("d1_ih", g, k), P)
    for g in range(3):
        for k in range(8):
            pk3.add(("d2_ih", g, k), P)
    if not zeros_h:
        for g in range(3):
            for k in range(8):
                pk3.add(("d1_hh", g, k), P)
        for g in range(3):
            for k in range(8):
                pk3.add(("d2_hh", g, k), P)
    for m in range(2):
        pk3.add(("outT", m), 80)
    return pk1, pk1b, pk3


_PACKS = {}


def get_packs(zeros_h):
    if zeros_h not in _PACKS:
        _PACKS[zeros_h] = _make_pack_layout(zeros_h)
    return _PACKS[zeros_h]

# bias pack column indices
BCOL = {
    "att_hsr": 2, "att_hsz": 3, "att_hbhhn": 4, "att_bihn": 5,
    "ld": 6,
    "d1_hsr": 7, "d1_hsz": 8, "d1_hbhhn": 9, "d1_bihn": 10,
    "d2_hsr": 11, "d2_hsz": 12, "d2_hbhhn": 13, "d2_bihn": 14,
    "bout0": 15, "battn": 16, "ones": 17, "bout1": 1,
}
for _m in range(8):
    BCOL[f"pre1m{_m}"] = 18 + _m
for _m in range(4):
    BCOL[f"pre2m{_m}"] = 26 + _m
for _m in range(8):
    BCOL[f"fr{_m}"] = 30 + _m
    BCOL[f"fz{_m}"] = 38 + _m
    BCOL[f"fbhhn{_m}"] = 46 + _m
    BCOL[f"fbihn{_m}"] = 54 + _m
    BCOL[f"fld{_m}"] = 62 + _m
NBCOL = 70


def _host_prep(inputs):
    """Returns (in_maps, perm, Ts, zeros_h) for the 8 cores."""
    zeros_h = all(
        not np.any(np.asarray(inputs[k]))
        for k in ("hidden_att", "hidden_dec1", "hidden_dec2"))
    lengths = np.asarray(inputs["lengths_enc"]).astype(np.int64)
    order = np.argsort(-lengths, kind="stable")
    # permuted batch column j = 8*c + s holds original row order[8*s + c]
    perm = np.empty(N, np.int64)
    for s in range(NSLOT):
        for c in range(NCORE):
            perm[NCORE * c + s] = order[NCORE * s + c]
    Ts = [int(math.ceil(int(lengths[order[NCORE * s]]) / P)) for s in range(NSLOT)]
    chunk_base = np.cumsum([0] + Ts)
    CH = int(chunk_base[-1])

    f32 = np.float32
    W = {k: np.asarray(v, dtype=np.float32) if np.asarray(v).dtype != np.int64 else v
         for k, v in inputs.items()}

    # batch-permuted, feature-major activations/aux (shared across cores)
    xdec = np.zeros((P, B), BF16NP)
    xdec[:O, :] = W["input_dec"][perm].T.astype(BF16NP)      # [80->128, 64]
    h0att_f32 = np.ascontiguousarray(W["hidden_att"][perm].T)  # [1024, 64]
    h0d1_f32 = np.ascontiguousarray(W["hidden_dec1"][perm].T)
    h0d2_f32 = np.ascontiguousarray(W["hidden_dec2"][perm].T)
    h0att = h0att_f32.astype(BF16NP)
    h0d1 = h0d1_f32.astype(BF16NP)
    h0d2 = h0d2_f32.astype(BF16NP)
    wb = np.ascontiguousarray(
        np.tile(W["W_attn"][0][None, :], (P, 1))).astype(BF16NP)  # [128,1024]

    PK1, PK1B, PK3 = get_packs(zeros_h)
    in_maps = []
    for c in range(NCORE):
        rowsP = slice(c * P, (c + 1) * P)

        blocks1 = {}
        for m in range(8):
            blocks1[("pre1", m)] = _lhsT(W["W_pre1"],
                                         slice(m * P, (m + 1) * P), 0)
        for m in range(4):
            for k in range(8):
                blocks1[("pre2", m, k)] = _lhsT(
                    W["W_pre2"], slice(m * P, (m + 1) * P), k)
        r64 = slice(c * 64, (c + 1) * 64)
        for g in range(3):
            rg = slice(g * H2 + c * P, g * H2 + (c + 1) * P)
            for k in range(4):
                blocks1[("att_ih", g, k)] = _lhsT(W["Wih_att"], rg, k)
            if not zeros_h:
                for k in range(8):
                    blocks1[("att_hh", g, k)] = _lhsT(W["Whh_att"], rg, k)
        for h in range(2):
            blk = W["W_ld"][h * 512 : (h + 1) * 512, rowsP].T
            blocks1[("ldT", h)] = np.ascontiguousarray(blk)
        wts1 = PK1.build(blocks1)
        blocks1b = {}
        if zeros_h:
            for g in range(3):
                for m in range(8):
                    rgm = slice(g * H2 + m * P, g * H2 + (m + 1) * P)
                    for k in range(4):
                        blocks1b[("ihf", g, m, k)] = _lhsT(W["Wih_att"],
                                                           rgm, k)
            for m in range(8):
                for k in range(8):
                    blocks1b[("ldf", m, k)] = _lhsT(
                        W["W_ld"], slice(m * P, (m + 1) * P), k)
        wts1b = PK1B.build(blocks1b) if zeros_h else None

        blocks3 = {}
        for k in range(16):
            blocks3[("sc", k)] = _lhsT(W["W_sc"], rowsP, k)
        for g in range(3):
            rg = slice(g * H2 + c * P, g * H2 + (c + 1) * P)
            for k in range(16):
                blocks3[("d1_ih", g, k)] = _lhsT(W["Wih_d1"], rg, k)
            for k in range(8):
                blocks3[("d2_ih", g, k)] = _lhsT(W["Wih_d2"], rg, k)
            if not zeros_h:
                for k in range(8):
                    blocks3[("d1_hh", g, k)] = _lhsT(W["Whh_d1"], rg, k)
                for k in range(8):
                    blocks3[("d2_hh", g, k)] = _lhsT(W["Whh_d2"], rg, k)
        for m in range(2):
            blk = W["W_out"][m * 80 : (m + 1) * 80, rowsP].T
            blocks3[("outT", m)] = np.ascontiguousarray(blk)
        wts3 = PK3.build(blocks3)

        # biases
        bias = np.zeros((P, NBCOL), f32)

        def bput(name, vec):
            v = np.asarray(vec, f32).ravel()
            bias[: v.shape[0], BCOL[name]] = v

        for m in range(8):
            bput(f"pre1m{m}", W["b_pre1"][m * P : (m + 1) * P])
        for m in range(4):
            bput(f"pre2m{m}", W["b_pre2"][m * P : (m + 1) * P])

        def gru_bias(pfx, bih, bhh):
            br_i, bz_i, bn_i = bih[:H2], bih[H2:2*H2], bih[2*H2:]
            br_h, bz_h, bn_h = bhh[:H2], bhh[H2:2*H2], bhh[2*H2:]
            sl = rowsP
            bput(pfx + "_hsr", 0.5 * (br_i[sl] + br_h[sl]))
            bput(pfx + "_hsz", 0.5 * (bz_i[sl] + bz_h[sl]))
            bput(pfx + "_hbhhn", 0.5 * bn_h[sl])
            bput(pfx + "_bihn", bn_i[sl])

        gru_bias("att", W["bih_att"], W["bhh_att"])
        gru_bias("d1", W["bih_d1"], W["bhh_d1"])
        gru_bias("d2", W["bih_d2"], W["bhh_d2"])
        if zeros_h:
            bih, bhh = W["bih_att"], W["bhh_att"]
            for m in range(8):
                sl = slice(m * P, (m + 1) * P)
                bput(f"fr{m}", 0.5 * (bih[:H2] + bhh[:H2])[sl])
                bput(f"fz{m}", 0.5 * (bih[H2:2*H2] + bhh[H2:2*H2])[sl])
                bput(f"fbhhn{m}", 0.5 * bhh[2*H2:][sl])
                bput(f"fbihn{m}", bih[2*H2:][sl])
                bput(f"fld{m}", W["b_ld"][sl])
        bput("bout0", W["b_out"][0:80] / NCORE)
        bput("bout1", W["b_out"][80:160] / NCORE)
        bias[:, BCOL["battn"]] = float(W["b_attn"][0])
        bias[:, BCOL["ones"]] = 1.0
        bldrm = np.ascontiguousarray(
            np.tile(W["b_ld"][None, :], (NSLOT, 1)).astype(f32))

        # halved initial hidden slices [128, 3*64]
        hhalf = np.zeros((P, 3 * B), f32)
        hhalf[:, 0:B] = 0.5 * h0att_f32[rowsP, :]
        hhalf[:, B:2*B] = 0.5 * h0d1_f32[rowsP, :]
        hhalf[:, 2*B:3*B] = 0.5 * h0d2_f32[rowsP, :]

        # ragged-packed big tensors (bf16) + mask
        attw_p = np.zeros((CH * P, H2), BF16NP)
        enc_p = np.zeros((CH * P, H2), BF16NP)
        mask = np.zeros((P, NSLOT * 8), f32)
        for s in range(NSLOT):
            row = int(order[NCORE * s + c])
            ln = int(lengths[row])
            nfull = min(Ts[s] * P, T)
            base = int(chunk_base[s]) * P
            attw_p[base : base + nfull] = \
                W["input_attW_enc"][row, :nfull].astype(BF16NP)
            enc_p[base : base + nfull] = \
                W["input_enc"][row, :nfull].astype(BF16NP)
            for cc in range(Ts[s]):
                lo = cc * P
                valid = np.clip(ln - lo, 0, P)
                mask[:valid, s * 8 + cc] = 1.0

        xdec_my = np.zeros((P, NSLOT), BF16NP)
        xdec_my[:O, :] = W["input_dec"][perm[c * NSLOT : (c + 1) * NSLOT]].T \
            .astype(BF16NP)

        in_maps.append({
            "wts1": wts1, "wts3": wts3, "bias": bias, "hhalf": hhalf,
            "xdec": xdec, "h0att": h0att, "h0d1": h0d1, "h0d2": h0d2,
            "wb": wb, "mask": mask, "attw": attw_p, "enc": enc_p,
            "bldrm": bldrm, "xdecmy": xdec_my,
        })
        if zeros_h:
            in_maps[-1]["wts1b"] = wts1b
    return in_maps, perm, Ts, zeros_h


# ---------------------------------------------------------------------------
# device program
# ---------------------------------------------------------------------------
def build_program(Ts, zeros_h=True, stage=3):
    chunk_base = np.cumsum([0] + list(Ts))
    CH = int(chunk_base[-1])
    PK1, PK1B, PK3 = get_packs(zeros_h)

    nc = bacc.Bacc("TRN2", target_bir_lowering=False, debug=False,
                   num_devices=NCORE)

    # I/O
    d_wts1 = nc.dram_tensor("wts1", [P, PK1.pos], BF16, kind="ExternalInput")
    d_wts1b = (nc.dram_tensor("wts1b", [P, PK1B.pos], BF16,
                              kind="ExternalInput") if zeros_h else None)
    d_wts3 = nc.dram_tensor("wts3", [P, PK3.pos], BF16, kind="ExternalInput")
    d_bias = nc.dram_tensor("bias", [P, NBCOL], F32, kind="ExternalInput")
    d_hhalf = nc.dram_tensor("hhalf", [P, 3 * B], F32, kind="ExternalInput")
    d_xdec = nc.dram_tensor("xdec", [P, B], BF16, kind="ExternalInput")
    d_h0att = nc.dram_tensor("h0att", [H2, B], BF16, kind="ExternalInput")
    d_h0d1 = nc.dram_tensor("h0d1", [H2, B], BF16, kind="ExternalInput")
    d_h0d2 = nc.dram_tensor("h0d2", [H2, B], BF16, kind="ExternalInput")
    d_wb = nc.dram_tensor("wb", [P, H2], BF16, kind="ExternalInput")
    d_mask = nc.dram_tensor("mask", [P, NSLOT * 8], F32, kind="ExternalInput")
    d_attw = nc.dram_tensor("attw", [CH * P, H2], BF16, kind="ExternalInput")
    d_enc = nc.dram_tensor("enc", [CH * P, H2], BF16, kind="ExternalInput")
    d_bldrm = nc.dram_tensor("bldrm", [NSLOT, H2], F32, kind="ExternalInput")
    d_xdecmy = nc.dram_tensor("xdecmy", [P, NSLOT], BF16, kind="ExternalInput")

    o_hatt = nc.dram_tensor("o_hatt", [P, B], F32, kind="ExternalOutput")
    o_hd1 = nc.dram_tensor("o_hd1", [P, B], F32, kind="ExternalOutput")
    o_hd2 = nc.dram_tensor("o_hd2", [P, B], F32, kind="ExternalOutput")
    o_out = nc.dram_tensor("o_out", [KO, B], F32, kind="ExternalOutput")

    rg = [list(range(NCORE))]

    with tile.TileContext(nc) as tc:
        with tc.tile_pool(name="const", bufs=1) as const, \
             tc.tile_pool(name="work", bufs=2) as work, \
             tc.tile_pool(name="dram", bufs=1, space="DRAM") as dram:

            # ---------------- constants ----------------
            wts1_sb = const.tile([P, PK1.pos], BF16)
            nc.sync.dma_start(out=wts1_sb, in_=d_wts1[:])
            bias_sb = const.tile([P, NBCOL], F32)
            nc.sync.dma_start(out=bias_sb, in_=d_bias[:])
            hhalf_sb = const.tile([P, 3 * B], F32)
            nc.sync.dma_start(out=hhalf_sb, in_=d_hhalf[:])
            xdec_sb = const.tile([P, B], BF16)
            nc.sync.dma_start(out=xdec_sb, in_=d_xdec[:])
            wb_sb = const.tile([P, H2], BF16)
            nc.sync.dma_start(out=wb_sb, in_=d_wb[:])
            mask_sb = const.tile([P, NSLOT * 8], F32)
            nc.sync.dma_start(out=mask_sb, in_=d_mask[:])
            ident = const.tile([P, P], F32)
            make_identity(nc, ident)
            if zeros_h:
                wts1b_sb = const.tile([P, PK1B.pos], BF16)
                nc.sync.dma_start(out=wts1b_sb, in_=d_wts1b[:])
            wts3_sb = const.tile([P, PK3.pos], BF16)

            def w1(key):
                if key in PK1.off:
                    o, m = PK1.off[key]
                    return wts1_sb[:, o : o + m]
                o, m = PK1B.off[key]
                return wts1b_sb[:, o : o + m]

            def w3(key):
                o, m = PK3.off[key]
                return wts3_sb[:, o : o + m]

            def bcol(name):
                i = BCOL[name]
                return bias_sb[:, i : i + 1]

            def load_k_tiles(src, n, tag, cast=False):
                # cast=True: f32 DRAM -> bf16 SBUF (SWDGE dtype cast)
                tiles = []
                for k in range(n):
                    t = const.tile([P, B], BF16, tag=f"{tag}{k}",
                                   name=f"{tag}{k}")
                    eng = nc.gpsimd if cast else nc.sync
                    eng.dma_start(out=t, in_=src[k * P : (k + 1) * P, :])
                    tiles.append(t)
                return tiles

            if zeros_h:
                h0att_t = h0d1_t = h0d2_t = None
            else:
                h0att_t = load_k_tiles(d_h0att, 8, "h0att")
                h0d1_t = load_k_tiles(d_h0d1, 8, "h0d1")
                h0d2_t = load_k_tiles(d_h0d2, 8, "h0d2")

            def mm_acc(psum_ap, pairs):
                nmm = len(pairs)
                for i, (lhsT, rhs) in enumerate(pairs):
                    nc.tensor.matmul(psum_ap, lhsT, rhs,
                                     start=(i == 0), stop=(i == nmm - 1))

            def gru_combine(psum_rz, psum_gin, psum_ghn, pfx, hhalf_ap):
                tr = work.tile([P, B], F32, tag="gru_tr", name="gru_tr")
                nc.scalar.activation(tr, psum_rz[:, 0:B], AF.Tanh,
                                     bias=bcol(pfx + "_hsr"), scale=0.5)
                tz = work.tile([P, B], F32, tag="gru_tz", name="gru_tz")
                nc.scalar.activation(tz, psum_rz[:, B : 2 * B], AF.Tanh,
                                     bias=bcol(pfx + "_hsz"), scale=0.5)
                if psum_ghn is None:
                    # h0 == 0: h_n is just bhh_n; use the halved bias column
                    hnh = bcol(pfx + "_hbhhn").broadcast_to([P, B])
                else:
                    hnh = work.tile([P, B], F32, tag="gru_hnh",
                                    name="gru_hnh")
                    nc.scalar.activation(hnh, psum_ghn, AF.Identity,
                                         bias=bcol(pfx + "_hbhhn"), scale=0.5)
                rn = work.tile([P, B], F32, tag="gru_rn", name="gru_rn")
                nc.vector.scalar_tensor_tensor(rn, tr, 1.0, hnh, ALU.add, ALU.mult)
                arg = work.tile([P, B], F32, tag="gru_arg", name="gru_arg")
                nc.vector.tensor_add(out=arg, in0=psum_gin, in1=rn)
                nn = work.tile([P, B], F32, tag="gru_n", name="gru_n")
                nc.scalar.activation(nn, arg, AF.Tanh, bias=bcol(pfx + "_bihn"))
                dh = work.tile([P, B], F32, tag="gru_dh", name="gru_dh")
                nc.vector.scalar_tensor_tensor(dh, nn, -0.5, hhalf_ap,
                                               ALU.mult, ALU.add)
                zd = work.tile([P, B], F32, tag="gru_zd", name="gru_zd")
                nc.vector.scalar_tensor_tensor(zd, tz, 1.0, dh, ALU.add, ALU.mult)
                hnew = work.tile([P, B], F32, tag=f"gru_h_{pfx}", name=f"gru_h_{pfx}")
                nc.vector.tensor_add(out=hnew, in0=nn, in1=zd)
                return hnew

            def allgather(in_sb, out_rows, tag):
                cin = dram.tile(list(in_sb.shape), F32, tag=f"agi_{tag}",
                                name=f"agi_{tag}")
                cout = dram.tile([out_rows, in_sb.shape[1]], F32,
                                 tag=f"ago_{tag}", name=f"ago_{tag}")
                nc.sync.dma_start(out=cin, in_=in_sb)
                nc.gpsimd.collective_compute(
                    "AllGather", ALU.bypass, replica_groups=rg,
                    ins=[cin.opt()], outs=[cout.opt()])
                return cout

            # ---------------- phase 1 ----------------
            with tc.tile_pool(name="ps1", bufs=1, space="PSUM") as ps1:
                dp_dram = dram.tile([NSLOT, H2], BF16, name="dp_dram")
                if zeros_h:
                    # Collective-free dec_proj: recompute prenet + GRU-att +
                    # W_ld for THIS core's 8 batch rows with replicated
                    # weights (h0 == 0, so no Whh terms). Row-major dp via
                    # 8 tiny PE transposes.
                    xdmy_sb = const.tile([P, NSLOT], BF16, name="xdmy_sb")
                    nc.sync.dma_start(out=xdmy_sb, in_=d_xdecmy[:])
                    pre1my = []
                    for m in range(8):
                        pm = ps1.tile([P, NSLOT], F32, tag="pp8", bufs=2, name="pm")
                        mm_acc(pm, [(w1(("pre1", m)), xdmy_sb)])
                        t = work.tile([P, NSLOT], BF16, tag=f"p1my{m}",
                                      name=f"p1my{m}")
                        nc.scalar.activation(t, pm, AF.Relu,
                                             bias=bcol(f"pre1m{m}"))
                        pre1my.append(t)
                    pre2my = []
                    for m in range(4):
                        pm = ps1.tile([P, NSLOT], F32, tag="pp8", bufs=2, name="pm2")
                        mm_acc(pm, [(w1(("pre2", m, k)), pre1my[k])
                                    for k in range(8)])
                        t = work.tile([P, NSLOT], BF16, tag=f"p2my{m}",
                                      name=f"p2my{m}")
                        nc.scalar.activation(t, pm, AF.Relu,
                                             bias=bcol(f"pre2m{m}"))
                        pre2my.append(t)
                    hmy = []
                    for m in range(8):
                        pg = ps1.tile([P, 3 * NSLOT], F32, tag="pg", bufs=2,
                                      name="pg")
                        for g in range(3):
                            mm_acc(pg[:, g * NSLOT : (g + 1) * NSLOT],
                                   [(w1(("ihf", g, m, k)), pre2my[k])
                                    for k in range(4)])
                        ftr = work.tile([P, NSLOT], F32, tag="ftr",
                                        name="ftr")
                        nc.scalar.activation(ftr, pg[:, 0:NSLOT], AF.Tanh,
                                             bias=bcol(f"fr{m}"), scale=0.5)
                        ftz = work.tile([P, NSLOT], F32, tag="ftz",
                                        name="ftz")
                        nc.scalar.activation(ftz, pg[:, NSLOT : 2 * NSLOT],
                                             AF.Tanh, bias=bcol(f"fz{m}"),
                                             scale=0.5)
                        frn = work.tile([P, NSLOT], F32, tag="frn",
                                        name="frn")
                        nc.vector.scalar_tensor_tensor(
                            frn, ftr, 1.0,
                            bcol(f"fbhhn{m}").broadcast_to([P, NSLOT]),
                            ALU.add, ALU.mult)
                        farg = work.tile([P, NSLOT], F32, tag="farg",
                                         name="farg")
                        nc.vector.tensor_add(out=farg,
                                             in0=pg[:, 2 * NSLOT :],
                                             in1=frn)
                        fn = work.tile([P, NSLOT], F32, tag="fn", name="fn")
                        nc.scalar.activation(fn, farg, AF.Tanh,
                                             bias=bcol(f"fbihn{m}"))
                        fdh = work.tile([P, NSLOT], F32, tag="fdh",
                                        name="fdh")
                        nc.vector.tensor_scalar_mul(fdh, fn, -0.5)
                        fzd = work.tile([P, NSLOT], F32, tag="fzd",
                                        name="fzd")
                        nc.vector.scalar_tensor_tensor(fzd, ftz, 1.0, fdh,
                                                       ALU.add, ALU.mult)
                        hm = work.tile([P, NSLOT], BF16, tag=f"hmy{m}",
                                       name=f"hmy{m}")
                        nc.vector.tensor_add(out=hm, in0=fn, in1=fzd)
                        hmy.append(hm)
                    # dp for my rows + transpose to row-major
                    pdprm = ps1.tile([NSLOT, H2], F32, tag="pdprm",
                                     name="pdprm")
                    for m in range(8):
                        pdp = ps1.tile([P, NSLOT], F32, tag="pp8", bufs=2,
                                       name="pdp")
                        mm_acc(pdp, [(w1(("ldf", m, k)), hmy[k])
                                     for k in range(8)])
                        dmy = work.tile([P, NSLOT], F32, tag="dmy",
                                        name="dmy")
                        nc.scalar.activation(dmy, pdp, AF.Identity,
                                             bias=bcol(f"fld{m}"))
                        nc.tensor.transpose(pdprm[:, m * P : (m + 1) * P],
                                            dmy, ident)
                    dprm = work.tile([NSLOT, H2], BF16, name="dprm")
                    nc.scalar.copy(dprm, pdprm)
                    nc.sync.dma_start(out=dp_dram, in_=dprm)

                # prenet replicated on every core (no collectives)
                pre1f = []
                for m in range(8):
                    p1 = ps1.tile([P, B], F32, tag="pg", bufs=2, name="p1")
                    mm_acc(p1, [(w1(("pre1", m)), xdec_sb)])
                    t = const.tile([P, B], BF16, tag=f"pre1f{m}",
                                   name=f"pre1f{m}")
                    nc.scalar.activation(t, p1, AF.Relu,
                                         bias=bcol(f"pre1m{m}"))
                    pre1f.append(t)
                pre2f = []
                for m in range(4):
                    p2 = ps1.tile([P, B], F32, tag="pg", bufs=2, name="p2")
                    mm_acc(p2, [(w1(("pre2", m, k)), pre1f[k])
                                for k in range(8)])
                    t = const.tile([P, B], BF16, tag=f"pre2f{m}",
                                   name=f"pre2f{m}")
                    nc.scalar.activation(t, p2, AF.Relu,
                                         bias=bcol(f"pre2m{m}"))
                    pre2f.append(t)

                # sharded GRU-att (h_att output + phase-3 AllGather)
                rz = ps1.tile([P, 2 * B], F32, tag="rz", name="rz")
                for g in (0, 1):
                    reg = rz[:, g * B : (g + 1) * B]
                    pairs = [(w1(("att_ih", g, k)), pre2f[k]) for k in range(4)]
                    if not zeros_h:
                        pairs += [(w1(("att_hh", g, k)), h0att_t[k])
                                  for k in range(8)]
                    mm_acc(reg, pairs)
                gin = ps1.tile([P, B], F32, tag="gin", name="gin")
                mm_acc(gin, [(w1(("att_ih", 2, k)), pre2f[k]) for k in range(4)])
                if zeros_h:
                    ghn = None
                else:
                    ghn = ps1.tile([P, B], F32, tag="ghn", name="ghn")
                    mm_acc(ghn, [(w1(("att_hh", 2, k)), h0att_t[k])
                                 for k in range(8)])
                hatt_c = gru_combine(rz, gin, ghn, "att", hhalf_sb[:, 0:B])
                nc.sync.dma_start(out=o_hatt[:], in_=hatt_c)

                if not zeros_h:
                    # dec_proj via K-sharded GEMM + ReduceScatter
                    hatt_bf = work.tile([P, B], BF16, name="hatt_bf")
                    nc.vector.tensor_copy(out=hatt_bf, in_=hatt_c)
                    pdpt = ps1.tile([B, H2], F32, tag="pdpt", name="pdpt")
                    for h in range(2):
                        nc.tensor.matmul(pdpt[:, h * 512 : (h + 1) * 512],
                                         hatt_bf, w1(("ldT", h)),
                                         start=True, stop=True)
                    dpt_sb = work.tile([B, H2], F32, name="dpt_sb")
                    nc.scalar.copy(dpt_sb, pdpt)
                    rs_in = dram.tile([B, H2], F32, name="rs_in")
                    nc.sync.dma_start(out=rs_in, in_=dpt_sb)
                    rs_out = dram.tile([NSLOT, H2], F32, name="rs_out")
                    nc.gpsimd.collective_compute(
                        "ReduceScatter", ALU.add, replica_groups=rg,
                        ins=[rs_in.opt()], outs=[rs_out.opt()])
                    dprm_f = work.tile([NSLOT, H2], F32, name="dprm_f")
                    nc.sync.dma_start(out=dprm_f, in_=rs_out[:])
                    bld_sb = work.tile([NSLOT, H2], F32, name="bld_sb")
                    nc.sync.dma_start(out=bld_sb, in_=d_bldrm[:])
                    dprm2 = work.tile([NSLOT, H2], BF16, name="dprm2")
                    nc.vector.tensor_add(out=dprm2, in0=dprm_f, in1=bld_sb)
                    nc.sync.dma_start(out=dp_dram, in_=dprm2)

            # NOTE: the h_att AllGather is deliberately emitted AFTER the
            # attention loop — collective triggers live on the in-order
            # GpSimd queue, and an earlier emission blocks the dps
            # broadcast DMAs behind the collective.

            # ---------------- phase 2: attention ----------------
            ag_at_in = dram.tile([NSLOT, H2], F32, name="ag_at_in")
            ag_at = dram.tile([N, H2], F32, name="ag_at")
            with tc.tile_pool(name="ps2", bufs=1, space="PSUM") as ps2:
                for s in range(NSLOT if stage >= 2 else 0):
                    # broadcast dp row s across 128 partitions
                    dps = work.tile([P, H2], BF16, tag="dps", bufs=3,
                                    name="dps")
                    row = dp_dram[s : s + 1, :]
                    bc = bass.AP(tensor=row.tensor, offset=row.offset,
                                 ap=[[0, P], row.ap[-1]])
                    nc.gpsimd.dma_start(out=dps, in_=bc)

                    score = work.tile([P, 8], F32, tag="score", name="score")
                    # two passes so DVE never stalls behind ACT's tanh:
                    # all adds (DVE) + tanhs (ACT) first, then the score
                    # reductions (DVE) once tanhs are draining.
                    ats = []
                    for cc in range(Ts[s]):
                        base = (int(chunk_base[s]) + cc) * P
                        at = work.tile([P, H2], BF16, tag="at", bufs=24,
                                       name="at")
                        nc.sync.dma_start(out=at,
                                          in_=d_attw[base : base + P, :])
                        nc.vector.tensor_add(out=at, in0=at, in1=dps)
                        nc.scalar.activation(at, at, AF.Tanh)
                        ats.append(at)
                    for cc, at in enumerate(ats):
                        nc.vector.scalar_tensor_tensor(
                            at, at, 0.0, wb_sb, ALU.add, ALU.mult,
                            accum_out=score[:, cc : cc + 1])

                    # exp, mask, row-sums
                    escore = work.tile([P, 8], F32, tag="escore", name="escore")
                    nc.scalar.activation(escore[:, 0 : Ts[s]],
                                         score[:, 0 : Ts[s]], AF.Exp,
                                         bias=bcol("battn"))
                    wt = work.tile([P, 8], BF16, tag="wt", name="wt")
                    psums = work.tile([P, 1], F32, tag="psums", name="psums")
                    nc.vector.scalar_tensor_tensor(
                        wt[:, 0 : Ts[s]], escore[:, 0 : Ts[s]], 0.0,
                        mask_sb[:, s * 8 : s * 8 + Ts[s]], ALU.add, ALU.mult,
                        accum_out=psums)
                    # denom = max(sum, 1e-12); rec = 1/denom
                    pd = ps2.tile([1, 1], F32, tag="pd", name="pd")
                    nc.tensor.matmul(pd, bcol("ones"), psums, start=True,
                                     stop=True)
                    dsb = work.tile([1, 1], F32, tag="dsb", name="dsb")
                    nc.scalar.copy(dsb, pd)
                    nc.vector.tensor_scalar_max(dsb, dsb, 1e-12)
                    rec = work.tile([1, 1], F32, tag="rec", name="rec")
                    nc.vector.reciprocal(rec, dsb)

                    # weighted sum over enc chunks
                    at0 = ps2.tile([1, 512], F32, tag="at0", bufs=2, name="at0")
                    at1 = ps2.tile([1, 512], F32, tag="at1", bufs=2, name="at1")
                    for cc in range(Ts[s]):
                        base = (int(chunk_base[s]) + cc) * P
                        ec = work.tile([P, H2], BF16, tag="ec", bufs=10,
                                       name="ec")
                        nc.sync.dma_start(out=ec,
                                          in_=d_enc[base : base + P, :])
                        st, sp = cc == 0, cc == Ts[s] - 1
                        nc.tensor.matmul(at0, wt[:, cc : cc + 1], ec[:, 0:512],
                                         start=st, stop=sp)
                        nc.tensor.matmul(at1, wt[:, cc : cc + 1], ec[:, 512:H2],
                                         start=st, stop=sp)
                    attn_s = work.tile([1, H2], F32, tag="attn_s", bufs=2,
                                       name="attn_s")
                    nc.scalar.activation(attn_s[:, 0:512], at0,
                                         AF.Copy, scale=rec[:, 0:1])
                    nc.scalar.activation(attn_s[:, 512:H2], at1,
                                         AF.Copy, scale=rec[:, 0:1])
                    nc.sync.dma_start(out=ag_at_in[s : s + 1, :], in_=attn_s)

            # h_att AllGather (phase-3 input), then attention AllGather
            ag_h = allgather(hatt_c, H2, "hatt")
            hattf = load_k_tiles(ag_h, 8, "hattf", cast=True)
            if stage >= 2:
                nc.gpsimd.collective_compute(
                    "AllGather", ALU.bypass, replica_groups=rg,
                    ins=[ag_at_in.opt()], outs=[ag_at.opt()])

            # ---------------- phase 3 ----------------
            nc.sync.dma_start(out=wts3_sb, in_=d_wts3[:])
            with tc.tile_pool(name="ps3", bufs=1, space="PSUM") as ps3:
              if stage >= 3:
                # feature-major attn tiles via PE transpose
                attn_nat = work.tile([N, H2], F32, name="attn_nat")
                nc.sync.dma_start(out=attn_nat, in_=ag_at[:])
                attnf = []
                for k in range(8):
                    ptp = ps3.tile([P, B], F32, tag="ptp", bufs=2, name="ptp")
                    nc.tensor.transpose(ptp, attn_nat[:, k * P : (k + 1) * P],
                                        ident[0:N, 0:N])
                    tkf = const.tile([P, B], BF16, tag=f"attnf{k}",
                                     name=f"attnf{k}")
                    nc.scalar.copy(tkf, ptp)
                    attnf.append(tkf)
                decin = attnf + hattf  # 16 K-tiles of [128, 64]

                psc = ps3.tile([P, B], F32, tag="psc", name="psc")
                mm_acc(psc, [(w3(("sc", k)), decin[k]) for k in range(16)])

                rz1 = ps3.tile([P, 2 * B], F32, tag="rz", name="rz1")
                for g in (0, 1):
                    reg = rz1[:, g * B : (g + 1) * B]
                    pairs = [(w3(("d1_ih", g, k)), decin[k]) for k in range(16)]
                    if not zeros_h:
                        pairs += [(w3(("d1_hh", g, k)), h0d1_t[k])
                                  for k in range(8)]
                    mm_acc(reg, pairs)
                gin1 = ps3.tile([P, B], F32, tag="gin", name="gin1")
                mm_acc(gin1, [(w3(("d1_ih", 2, k)), decin[k]) for k in range(16)])
                if zeros_h:
                    ghn1 = None
                else:
                    ghn1 = ps3.tile([P, B], F32, tag="ghn", name="ghn1")
                    mm_acc(ghn1, [(w3(("d1_hh", 2, k)), h0d1_t[k])
                                  for k in range(8)])
                hd1_c = gru_combine(rz1, gin1, ghn1, "d1",
                                    hhalf_sb[:, B : 2 * B])
                nc.sync.dma_start(out=o_hd1[:], in_=hd1_c)

                r2_c = work.tile([P, B], F32, name="r2_c")
                nc.vector.tensor_add(out=r2_c, in0=psc, in1=hd1_c)
                ag_r2 = allgather(r2_c, H2, "r2")
                r2f = load_k_tiles(ag_r2, 8, "r2f", cast=True)

                rz2 = ps3.tile([P, 2 * B], F32, tag="rz", name="rz2")
                for g in (0, 1):
                    reg = rz2[:, g * B : (g + 1) * B]
                    pairs = [(w3(("d2_ih", g, k)), r2f[k]) for k in range(8)]
                    if not zeros_h:
                        pairs += [(w3(("d2_hh", g, k)), h0d2_t[k])
                                  for k in range(8)]
                    mm_acc(reg, pairs)
                gin2 = ps3.tile([P, B], F32, tag="gin", name="gin2")
                mm_acc(gin2, [(w3(("d2_ih", 2, k)), r2f[k]) for k in range(8)])
                if zeros_h:
                    ghn2 = None
                else:
                    ghn2 = ps3.tile([P, B], F32, tag="ghn", name="ghn2")
                    mm_acc(ghn2, [(w3(("d2_hh", 2, k)), h0d2_t[k])
                                  for k in range(8)])
                hd2_c = gru_combine(rz2, gin2, ghn2, "d2",
                                    hhalf_sb[:, 2 * B : 3 * B])
                nc.sync.dma_start(out=o_hd2[:], in_=hd2_c)

                r3_c = work.tile([P, B], F32, name="r3_c")
                nc.vector.tensor_add(out=r3_c, in0=r2_c, in1=hd2_c)

                # output GEMM, K-sharded + AllReduce (bias pre-divided by 8)
                r3bf = work.tile([P, B], BF16, name="r3bf")
                nc.vector.tensor_copy(out=r3bf, in_=r3_c)
                ar_in = dram.tile([KO, B], F32, name="ar_in")
                for m in range(2):
                    pout = ps3.tile([80, B], F32, tag="pout", name="pout")
                    nc.tensor.matmul(pout, w3(("outT", m)), r3bf,
                                     start=True, stop=True)
                    osb = work.tile([80, B], F32, tag="osb", name="osb")
                    nc.scalar.activation(osb, pout, AF.Identity,
                                         bias=bcol(f"bout{m}")[0:80])
                    nc.sync.dma_start(out=ar_in[m * 80 : (m + 1) * 80, :],
                                      in_=osb)
                ar_out = dram.tile([KO, B], F32, name="ar_out")
                nc.gpsimd.collective_compute(
                    "AllReduce", ALU.add, replica_groups=rg,
                    ins=[ar_in.opt()], outs=[ar_out.opt()])
                nc.sync.dma_start(out=o_out[:], in_=ar_out[:])

    nc.compile()
    return nc


# ---------------------------------------------------------------------------
# entry point
# ---------------------------------------------------------------------------
_PROGRAM_CACHE = {}


def _get_program(Ts, zeros_h):
    key = (tuple(Ts), zeros_h)
    if key not in _PROGRAM_CACHE:
        _PROGRAM_CACHE[key] = build_program(Ts, zeros_h)
    return _PROGRAM_CACHE[key]


def run_device(inputs, trace=False, **kw):
    in_maps, perm, Ts, zeros_h = _host_prep(inputs)
    nc = _get_program(Ts, zeros_h)
    res = run_bass_kernel_spmd(nc, in_maps, list(range(NCORE)), trace=trace,
                               **kw)
    return res, perm


def _assemble(results, perm):
    inv = np.empty(N, np.int64)
    inv[perm] = np.arange(N)

    def gather(name, rows):
        full = np.concatenate([results[c][name][:rows] for c in range(NCORE)],
                              axis=0)
        return np.ascontiguousarray(full.T[inv])

    h_att = gather("o_hatt", P)
    h_d1 = gather("o_hd1", P)
    h_d2 = gather("o_hd2", P)
    out = np.ascontiguousarray(results[0]["o_out"].T[inv]).reshape(N, R, O)
    return out, h_att, h_d1, h_d2


def kernel(**inputs):
    res, perm = run_device(inputs)
    return _assemble(res.results, perm)


# revision 73
# speedup vs baseline: 1.0522x; 1.0522x over previous
"""Trainium2 Bass kernel for AttnDecoderRNN single-step (N=64, T=800, H=512).

Strategy (8 NeuronCores, SPMD single program):
  - Weight GEMM phases are model-parallel: each core owns a 1/8 feature slice
    of every weight matrix; activations stay feature-major [feat, batch=64];
    AllGather (partition-axis concat) rebuilds full activations between GEMMs.
  - The large attention middle (e = tanh(attW + dec_proj), score, weighted
    sum over T with ragged length masking) is batch-parallel: rows are sorted
    by length and dealt round-robin to cores so every "row slot" s has a
    uniform compile-time chunk count T_s; per-core raggedness lives entirely
    in the (per-core) input data, keeping the program identical across cores.
  - dec_proj: when the initial hidden states are all-zero (guaranteed by the
    input spec, checked at runtime with a collective fallback), each core
    recomputes prenet+GRU-att+W_ld for its own 8 rows with replicated bf16
    weights — no collective on the critical path; otherwise a K-sharded GEMM
    + ReduceScatter delivers each core its rows. All GEMMs run in bf16
    (fp32 matmul is a 2-pass LOW_HIGH mode, ~4x slower); accumulation and
    the softmax-like score path stay fp32.
Host side packs weight slices / biases / ragged-packed bf16 attW & enc
buffers per core, and reassembles + unpermutes outputs.
"""

import math
import sys

for _p in ("/opt/trn_rl_repo",):
    if _p not in sys.path:
        sys.path.insert(0, _p)

import ml_dtypes
import numpy as np

BF16NP = ml_dtypes.bfloat16

import concourse.bass as bass
import concourse.mybir as mybir
import concourse.tile as tile
from concourse import bacc
from concourse.bass_utils import run_bass_kernel_spmd
from concourse.masks import make_identity

F32 = mybir.dt.float32
BF16 = mybir.dt.bfloat16
AF = mybir.ActivationFunctionType
ALU = mybir.AluOpType

N, T, H, O, R = 64, 800, 512, 80, 2
H2 = 2 * H            # 1024
KO = R * O            # 160
P = 128               # partitions
NCORE = 8
B = N                 # batch (free dim in GEMM phases)
NSLOT = N // NCORE    # 8 row slots per core


# ---------------------------------------------------------------------------
# host-side weight packing
# ---------------------------------------------------------------------------
class Pack:
    """Concatenate [128, m] blocks along the free dim; identical key->offset
    mapping on every core (shapes are core-independent)."""

    def __init__(self):
        self.off = {}
        self.pos = 0
        self.order = []

    def add(self, key, m):
        assert key not in self.off
        self.off[key] = (self.pos, m)
        self.order.append(key)
        self.pos += m

    def build(self, blocks):
        out = np.zeros((P, self.pos), BF16NP)
        for key, arr in blocks.items():
            o, m = self.off[key]
            assert arr.shape == (P, m), (key, arr.shape, m)
            out[:, o : o + m] = arr.astype(BF16NP)
        return out


def _lhsT(w, rows, kchunk):
    """w[rows, 128*kchunk : 128*(kchunk+1)].T as [128, len(rows)] (K zero-pad)."""
    blk = w[rows, P * kchunk : P * (kchunk + 1)].T.astype(np.float32)
    if blk.shape[0] < P:
        blk = np.pad(blk, ((0, P - blk.shape[0]), (0, 0)))
    return np.ascontiguousarray(blk)


def _make_pack_layout(zeros_h):
    """zeros_h: initial hidden states are all-zero (guaranteed by the input
    spec); drops every Whh @ h0 GEMM and adds replicated Wih_att/W_ld for the
    collective-free dec_proj fast path."""
    pk1 = Pack()
    for m in range(8):          # prenet replicated: full output dims
        pk1.add(("pre1", m), P)
    for m in range(4):
        for k in range(8):
            pk1.add(("pre2", m, k), P)
    for g in range(3):
        for k in range(4):
            pk1.add(("att_ih", g, k), P)
    if not zeros_h:
        for g in range(3):
            for k in range(8):
                pk1.add(("att_hh", g, k), P)
    for h in range(2):
        pk1.add(("ldT", h), 512)
    # replicated fast-path weights in their own tensor so its DMA is staged
    pk1b = Pack()
    if zeros_h:
        for g in range(3):
            for m in range(8):
                for k in range(4):
                    pk1b.add(("ihf", g, m, k), P)
        for m in range(8):
            for k in range(8):
                pk1b.add(("ldf", m, k), P)

    pk3 = Pack()
    for k in range(16):
        pk3.add(("sc", k), P)
    for g in range(3):
        for k in range(16):
            pk3.add(("d1_ih", g, k), P)
    for g in range(3):
        for k in range(8):
            pk3.add(("d2_ih", g, k), P)
    if not zeros_h:
        for g in range(3):
            for k in range(8):
                pk3.add(("d1_hh", g, k), P)
        for g in range(3):
            for k in range(8):
                pk3.add(("d2_hh", g, k), P)
    for m in range(2):
        pk3.add(("outT", m), 80)
    return pk1, pk1b, pk3


_PACKS = {}


def get_packs(zeros_h):
    if zeros_h not in _PACKS:
        _PACKS[zeros_h] = _make_pack_layout(zeros_h)
    return _PACKS[zeros_h]

# bias pack column indices
BCOL = {
    "att_hsr": 2, "att_hsz": 3, "att_hbhhn": 4, "att_bihn": 5,
    "ld": 6,
    "d1_hsr": 7, "d1_hsz": 8, "d1_hbhhn": 9, "d1_bihn": 10,
    "d2_hsr": 11, "d2_hsz": 12, "d2_hbhhn": 13, "d2_bihn": 14,
    "bout0": 15, "battn": 16, "ones": 17, "bout1": 1,
}
for _m in range(8):
    BCOL[f"pre1m{_m}"] = 18 + _m
for _m in range(4):
    BCOL[f"pre2m{_m}"] = 26 + _m
for _m in range(8):
    BCOL[f"fr{_m}"] = 30 + _m
    BCOL[f"fz{_m}"] = 38 + _m
    BCOL[f"fbhhn{_m}"] = 46 + _m
    BCOL[f"fbihn{_m}"] = 54 + _m
    BCOL[f"fld{_m}"] = 62 + _m
NBCOL = 70


def _host_prep(inputs):
    """Returns (in_maps, perm, Ts, zeros_h) for the 8 cores."""
    zeros_h = all(
        not np.any(np.asarray(inputs[k]))
        for k in ("hidden_att", "hidden_dec1", "hidden_dec2"))
    lengths = np.asarray(inputs["lengths_enc"]).astype(np.int64)
    order = np.argsort(-lengths, kind="stable")
    # permuted batch column j = 8*c + s holds original row order[8*s + c]
    perm = np.empty(N, np.int64)
    for s in range(NSLOT):
        for c in range(NCORE):
            perm[NCORE * c + s] = order[NCORE * s + c]
    Ts = [int(math.ceil(int(lengths[order[NCORE * s]]) / P)) for s in range(NSLOT)]
    chunk_base = np.cumsum([0] + Ts)
    CH = int(chunk_base[-1])

    f32 = np.float32
    W = {k: np.asarray(v, dtype=np.float32) if np.asarray(v).dtype != np.int64 else v
         for k, v in inputs.items()}

    # batch-permuted, feature-major activations/aux (shared across cores)
    xdec = np.zeros((P, B), BF16NP)
    xdec[:O, :] = W["input_dec"][perm].T.astype(BF16NP)      # [80->128, 64]
    h0att_f32 = np.ascontiguousarray(W["hidden_att"][perm].T)  # [1024, 64]
    h0d1_f32 = np.ascontiguousarray(W["hidden_dec1"][perm].T)
    h0d2_f32 = np.ascontiguousarray(W["hidden_dec2"][perm].T)
    h0att = h0att_f32.astype(BF16NP)
    h0d1 = h0d1_f32.astype(BF16NP)
    h0d2 = h0d2_f32.astype(BF16NP)
    wb = np.ascontiguousarray(
        np.tile(W["W_attn"][0][None, :], (P, 1))).astype(BF16NP)  # [128,1024]

    PK1, PK1B, PK3 = get_packs(zeros_h)
    in_maps = []
    for c in range(NCORE):
        rowsP = slice(c * P, (c + 1) * P)

        blocks1 = {}
        for m in range(8):
            blocks1[("pre1", m)] = _lhsT(W["W_pre1"],
                                         slice(m * P, (m + 1) * P), 0)
        for m in range(4):
            for k in range(8):
                blocks1[("pre2", m, k)] = _lhsT(
                    W["W_pre2"], slice(m * P, (m + 1) * P), k)
        r64 = slice(c * 64, (c + 1) * 64)
        for g in range(3):
            rg = slice(g * H2 + c * P, g * H2 + (c + 1) * P)
            for k in range(4):
                blocks1[("att_ih", g, k)] = _lhsT(W["Wih_att"], rg, k)
            if not zeros_h:
                for k in range(8):
                    blocks1[("att_hh", g, k)] = _lhsT(W["Whh_att"], rg, k)
        for h in range(2):
            blk = W["W_ld"][h * 512 : (h + 1) * 512, rowsP].T
            blocks1[("ldT", h)] = np.ascontiguousarray(blk)
        wts1 = PK1.build(blocks1)
        blocks1b = {}
        if zeros_h:
            for g in range(3):
                for m in range(8):
                    rgm = slice(g * H2 + m * P, g * H2 + (m + 1) * P)
                    for k in range(4):
                        blocks1b[("ihf", g, m, k)] = _lhsT(W["Wih_att"],
                                                           rgm, k)
            for m in range(8):
                for k in range(8):
                    blocks1b[("ldf", m, k)] = _lhsT(
                        W["W_ld"], slice(m * P, (m + 1) * P), k)
        wts1b = PK1B.build(blocks1b) if zeros_h else None

        blocks3 = {}
        for k in range(16):
            blocks3[("sc", k)] = _lhsT(W["W_sc"], rowsP, k)
        for g in range(3):
            rg = slice(g * H2 + c * P, g * H2 + (c + 1) * P)
            for k in range(16):
                blocks3[("d1_ih", g, k)] = _lhsT(W["Wih_d1"], rg, k)
            for k in range(8):
                blocks3[("d2_ih", g, k)] = _lhsT(W["Wih_d2"], rg, k)
            if not zeros_h:
                for k in range(8):
                    blocks3[("d1_hh", g, k)] = _lhsT(W["Whh_d1"], rg, k)
                for k in range(8):
                    blocks3[("d2_hh", g, k)] = _lhsT(W["Whh_d2"], rg, k)
        for m in range(2):
            blk = W["W_out"][m * 80 : (m + 1) * 80, rowsP].T
            blocks3[("outT", m)] = np.ascontiguousarray(blk)
        wts3 = PK3.build(blocks3)

        # biases
        bias = np.zeros((P, NBCOL), f32)

        def bput(name, vec):
            v = np.asarray(vec, f32).ravel()
            bias[: v.shape[0], BCOL[name]] = v

        for m in range(8):
            bput(f"pre1m{m}", W["b_pre1"][m * P : (m + 1) * P])
        for m in range(4):
            bput(f"pre2m{m}", W["b_pre2"][m * P : (m + 1) * P])

        def gru_bias(pfx, bih, bhh):
            br_i, bz_i, bn_i = bih[:H2], bih[H2:2*H2], bih[2*H2:]
            br_h, bz_h, bn_h = bhh[:H2], bhh[H2:2*H2], bhh[2*H2:]
            sl = rowsP
            bput(pfx + "_hsr", 0.5 * (br_i[sl] + br_h[sl]))
            bput(pfx + "_hsz", 0.5 * (bz_i[sl] + bz_h[sl]))
            bput(pfx + "_hbhhn", 0.5 * bn_h[sl])
            bput(pfx + "_bihn", bn_i[sl])

        gru_bias("att", W["bih_att"], W["bhh_att"])
        gru_bias("d1", W["bih_d1"], W["bhh_d1"])
        gru_bias("d2", W["bih_d2"], W["bhh_d2"])
        if zeros_h:
            bih, bhh = W["bih_att"], W["bhh_att"]
            for m in range(8):
                sl = slice(m * P, (m + 1) * P)
                bput(f"fr{m}", 0.5 * (bih[:H2] + bhh[:H2])[sl])
                bput(f"fz{m}", 0.5 * (bih[H2:2*H2] + bhh[H2:2*H2])[sl])
                bput(f"fbhhn{m}", 0.5 * bhh[2*H2:][sl])
                bput(f"fbihn{m}", bih[2*H2:][sl])
                bput(f"fld{m}", W["b_ld"][sl])
        bput("bout0", W["b_out"][0:80] / NCORE)
        bput("bout1", W["b_out"][80:160] / NCORE)
        bias[:, BCOL["battn"]] = float(W["b_attn"][0])
        bias[:, BCOL["ones"]] = 1.0
        bldrm = np.ascontiguousarray(
            np.tile(W["b_ld"][None, :], (NSLOT, 1)).astype(f32))

        # halved initial hidden slices [128, 3*64]
        hhalf = np.zeros((P, 3 * B), f32)
        hhalf[:, 0:B] = 0.5 * h0att_f32[rowsP, :]
        hhalf[:, B:2*B] = 0.5 * h0d1_f32[rowsP, :]
        hhalf[:, 2*B:3*B] = 0.5 * h0d2_f32[rowsP, :]

        # ragged-packed big tensors (bf16) + mask
        attw_p = np.zeros((CH * P, H2), BF16NP)
        enc_p = np.zeros((CH * P, H2), BF16NP)
        mask = np.zeros((P, NSLOT * 8), f32)
        for s in range(NSLOT):
            row = int(order[NCORE * s + c])
            ln = int(lengths[row])
            nfull = min(Ts[s] * P, T)
            base = int(chunk_base[s]) * P
            attw_p[base : base + nfull] = \
                W["input_attW_enc"][row, :nfull].astype(BF16NP)
            enc_p[base : base + nfull] = \
                W["input_enc"][row, :nfull].astype(BF16NP)
            for cc in range(Ts[s]):
                lo = cc * P
                valid = np.clip(ln - lo, 0, P)
                mask[:valid, s * 8 + cc] = 1.0

        xdec_my = np.zeros((P, NSLOT), BF16NP)
        xdec_my[:O, :] = W["input_dec"][perm[c * NSLOT : (c + 1) * NSLOT]].T \
            .astype(BF16NP)

        in_maps.append({
            "wts1": wts1, "wts3": wts3, "bias": bias, "hhalf": hhalf,
            "xdec": xdec, "h0att": h0att, "h0d1": h0d1, "h0d2": h0d2,
            "wb": wb, "mask": mask, "attw": attw_p, "enc": enc_p,
            "bldrm": bldrm, "xdecmy": xdec_my,
        })
        if zeros_h:
            in_maps[-1]["wts1b"] = wts1b
    return in_maps, perm, Ts, zeros_h


# ---------------------------------------------------------------------------
# device program
# ---------------------------------------------------------------------------
def build_program(Ts, zeros_h=True, stage=3):
    chunk_base = np.cumsum([0] + list(Ts))
    CH = int(chunk_base[-1])
    PK1, PK1B, PK3 = get_packs(zeros_h)

    nc = bacc.Bacc("TRN2", target_bir_lowering=False, debug=False,
                   num_devices=NCORE)

    # I/O
    d_wts1 = nc.dram_tensor("wts1", [P, PK1.pos], BF16, kind="ExternalInput")
    d_wts1b = (nc.dram_tensor("wts1b", [P, PK1B.pos], BF16,
                              kind="ExternalInput") if zeros_h else None)
    d_wts3 = nc.dram_tensor("wts3", [P, PK3.pos], BF16, kind="ExternalInput")
    d_bias = nc.dram_tensor("bias", [P, NBCOL], F32, kind="ExternalInput")
    d_hhalf = nc.dram_tensor("hhalf", [P, 3 * B], F32, kind="ExternalInput")
    d_xdec = nc.dram_tensor("xdec", [P, B], BF16, kind="ExternalInput")
    d_h0att = nc.dram_tensor("h0att", [H2, B], BF16, kind="ExternalInput")
    d_h0d1 = nc.dram_tensor("h0d1", [H2, B], BF16, kind="ExternalInput")
    d_h0d2 = nc.dram_tensor("h0d2", [H2, B], BF16, kind="ExternalInput")
    d_wb = nc.dram_tensor("wb", [P, H2], BF16, kind="ExternalInput")
    d_mask = nc.dram_tensor("mask", [P, NSLOT * 8], F32, kind="ExternalInput")
    d_attw = nc.dram_tensor("attw", [CH * P, H2], BF16, kind="ExternalInput")
    d_enc = nc.dram_tensor("enc", [CH * P, H2], BF16, kind="ExternalInput")
    d_bldrm = nc.dram_tensor("bldrm", [NSLOT, H2], F32, kind="ExternalInput")
    d_xdecmy = nc.dram_tensor("xdecmy", [P, NSLOT], BF16, kind="ExternalInput")

    o_hatt = nc.dram_tensor("o_hatt", [P, B], F32, kind="ExternalOutput")
    o_hd1 = nc.dram_tensor("o_hd1", [P, B], F32, kind="ExternalOutput")
    o_hd2 = nc.dram_tensor("o_hd2", [P, B], F32, kind="ExternalOutput")
    o_out = nc.dram_tensor("o_out", [KO, B], F32, kind="ExternalOutput")

    rg = [list(range(NCORE))]

    with tile.TileContext(nc) as tc:
        with tc.tile_pool(name="const", bufs=1) as const, \
             tc.tile_pool(name="work", bufs=2) as work, \
             tc.tile_pool(name="dram", bufs=1, space="DRAM") as dram:

            # ---------------- constants ----------------
            wts1_sb = const.tile([P, PK1.pos], BF16)
            nc.sync.dma_start(out=wts1_sb, in_=d_wts1[:])
            bias_sb = const.tile([P, NBCOL], F32)
            nc.sync.dma_start(out=bias_sb, in_=d_bias[:])
            hhalf_sb = const.tile([P, 3 * B], F32)
            nc.sync.dma_start(out=hhalf_sb, in_=d_hhalf[:])
            xdec_sb = const.tile([P, B], BF16)
            nc.sync.dma_start(out=xdec_sb, in_=d_xdec[:])
            wb_sb = const.tile([P, H2], BF16)
            nc.sync.dma_start(out=wb_sb, in_=d_wb[:])
            mask_sb = const.tile([P, NSLOT * 8], F32)
            nc.sync.dma_start(out=mask_sb, in_=d_mask[:])
            ident = const.tile([P, P], F32)
            make_identity(nc, ident)
            if zeros_h:
                wts1b_sb = const.tile([P, PK1B.pos], BF16)
                nc.sync.dma_start(out=wts1b_sb, in_=d_wts1b[:])
            wts3_sb = const.tile([P, PK3.pos], BF16)

            def w1(key):
                if key in PK1.off:
                    o, m = PK1.off[key]
                    return wts1_sb[:, o : o + m]
                o, m = PK1B.off[key]
                return wts1b_sb[:, o : o + m]

            def w3(key):
                o, m = PK3.off[key]
                return wts3_sb[:, o : o + m]

            def bcol(name):
                i = BCOL[name]
                return bias_sb[:, i : i + 1]

            def load_k_tiles(src, n, tag, cast=False):
                # cast=True: f32 DRAM -> bf16 SBUF (SWDGE dtype cast)
                tiles = []
                for k in range(n):
                    t = const.tile([P, B], BF16, tag=f"{tag}{k}",
                                   name=f"{tag}{k}")
                    eng = nc.gpsimd if cast else nc.sync
                    eng.dma_start(out=t, in_=src[k * P : (k + 1) * P, :])
                    tiles.append(t)
                return tiles

            if zeros_h:
                h0att_t = h0d1_t = h0d2_t = None
            else:
                h0att_t = load_k_tiles(d_h0att, 8, "h0att")
                h0d1_t = load_k_tiles(d_h0d1, 8, "h0d1")
                h0d2_t = load_k_tiles(d_h0d2, 8, "h0d2")

            def mm_acc(psum_ap, pairs):
                nmm = len(pairs)
                for i, (lhsT, rhs) in enumerate(pairs):
                    nc.tensor.matmul(psum_ap, lhsT, rhs,
                                     start=(i == 0), stop=(i == nmm - 1))

            def gru_combine(psum_rz, psum_gin, psum_ghn, pfx, hhalf_ap):
                tr = work.tile([P, B], F32, tag="gru_tr", name="gru_tr")
                nc.scalar.activation(tr, psum_rz[:, 0:B], AF.Tanh,
                                     bias=bcol(pfx + "_hsr"), scale=0.5)
                tz = work.tile([P, B], F32, tag="gru_tz", name="gru_tz")
                nc.scalar.activation(tz, psum_rz[:, B : 2 * B], AF.Tanh,
                                     bias=bcol(pfx + "_hsz"), scale=0.5)
                if psum_ghn is None:
                    # h0 == 0: h_n is just bhh_n; use the halved bias column
                    hnh = bcol(pfx + "_hbhhn").broadcast_to([P, B])
                else:
                    hnh = work.tile([P, B], F32, tag="gru_hnh",
                                    name="gru_hnh")
                    nc.scalar.activation(hnh, psum_ghn, AF.Identity,
                                         bias=bcol(pfx + "_hbhhn"), scale=0.5)
                rn = work.tile([P, B], F32, tag="gru_rn", name="gru_rn")
                nc.vector.scalar_tensor_tensor(rn, tr, 1.0, hnh, ALU.add, ALU.mult)
                arg = work.tile([P, B], F32, tag="gru_arg", name="gru_arg")
                nc.vector.tensor_add(out=arg, in0=psum_gin, in1=rn)
                nn = work.tile([P, B], F32, tag="gru_n", name="gru_n")
                nc.scalar.activation(nn, arg, AF.Tanh, bias=bcol(pfx + "_bihn"))
                dh = work.tile([P, B], F32, tag="gru_dh", name="gru_dh")
                nc.vector.scalar_tensor_tensor(dh, nn, -0.5, hhalf_ap,
                                               ALU.mult, ALU.add)
                zd = work.tile([P, B], F32, tag="gru_zd", name="gru_zd")
                nc.vector.scalar_tensor_tensor(zd, tz, 1.0, dh, ALU.add, ALU.mult)
                hnew = work.tile([P, B], F32, tag=f"gru_h_{pfx}", name=f"gru_h_{pfx}")
                nc.vector.tensor_add(out=hnew, in0=nn, in1=zd)
                return hnew

            def allgather(in_sb, out_rows, tag):
                cin = dram.tile(list(in_sb.shape), F32, tag=f"agi_{tag}",
                                name=f"agi_{tag}")
                cout = dram.tile([out_rows, in_sb.shape[1]], F32,
                                 tag=f"ago_{tag}", name=f"ago_{tag}")
                nc.sync.dma_start(out=cin, in_=in_sb)
                nc.gpsimd.collective_compute(
                    "AllGather", ALU.bypass, replica_groups=rg,
                    ins=[cin.opt()], outs=[cout.opt()])
                return cout

            # ---------------- phase 1 ----------------
            with tc.tile_pool(name="ps1", bufs=1, space="PSUM") as ps1:
                dp_dram = dram.tile([NSLOT, H2], BF16, name="dp_dram")
                if zeros_h:
                    # Collective-free dec_proj: recompute prenet + GRU-att +
                    # W_ld for THIS core's 8 batch rows with replicated
                    # weights (h0 == 0, so no Whh terms). Row-major dp via
                    # 8 tiny PE transposes.
                    xdmy_sb = const.tile([P, NSLOT], BF16, name="xdmy_sb")
                    nc.sync.dma_start(out=xdmy_sb, in_=d_xdecmy[:])
                    pre1my = []
                    for m in range(8):
                        pm = ps1.tile([P, NSLOT], F32, tag="pp8", bufs=2, name="pm")
                        mm_acc(pm, [(w1(("pre1", m)), xdmy_sb)])
                        t = work.tile([P, NSLOT], BF16, tag=f"p1my{m}",
                                      name=f"p1my{m}")
                        nc.scalar.activation(t, pm, AF.Relu,
                                             bias=bcol(f"pre1m{m}"))
                        pre1my.append(t)
                    pre2my = []
                    for m in range(4):
                        pm = ps1.tile([P, NSLOT], F32, tag="pp8", bufs=2, name="pm2")
                        mm_acc(pm, [(w1(("pre2", m, k)), pre1my[k])
                                    for k in range(8)])
                        t = work.tile([P, NSLOT], BF16, tag=f"p2my{m}",
                                      name=f"p2my{m}")
                        nc.scalar.activation(t, pm, AF.Relu,
                                             bias=bcol(f"pre2m{m}"))
                        pre2my.append(t)
                    hmy = []
                    for m in range(8):
                        pg = ps1.tile([P, 3 * NSLOT], F32, tag="pg", bufs=2,
                                      name="pg")
                        for g in range(3):
                            mm_acc(pg[:, g * NSLOT : (g + 1) * NSLOT],
                                   [(w1(("ihf", g, m, k)), pre2my[k])
                                    for k in range(4)])
                        ftr = work.tile([P, NSLOT], F32, tag="ftr",
                                        name="ftr")
                        nc.scalar.activation(ftr, pg[:, 0:NSLOT], AF.Tanh,
                                             bias=bcol(f"fr{m}"), scale=0.5)
                        ftz = work.tile([P, NSLOT], F32, tag="ftz",
                                        name="ftz")
                        nc.scalar.activation(ftz, pg[:, NSLOT : 2 * NSLOT],
                                             AF.Tanh, bias=bcol(f"fz{m}"),
                                             scale=0.5)
                        frn = work.tile([P, NSLOT], F32, tag="frn",
                                        name="frn")
                        nc.vector.scalar_tensor_tensor(
                            frn, ftr, 1.0,
                            bcol(f"fbhhn{m}").broadcast_to([P, NSLOT]),
                            ALU.add, ALU.mult)
                        farg = work.tile([P, NSLOT], F32, tag="farg",
                                         name="farg")
                        nc.vector.tensor_add(out=farg,
                                             in0=pg[:, 2 * NSLOT :],
                                             in1=frn)
                        fn = work.tile([P, NSLOT], F32, tag="fn", name="fn")
                        nc.scalar.activation(fn, farg, AF.Tanh,
                                             bias=bcol(f"fbihn{m}"))
                        fdh = work.tile([P, NSLOT], F32, tag="fdh",
                                        name="fdh")
                        nc.vector.tensor_scalar_mul(fdh, fn, -0.5)
                        fzd = work.tile([P, NSLOT], F32, tag="fzd",
                                        name="fzd")
                        nc.vector.scalar_tensor_tensor(fzd, ftz, 1.0, fdh,
                                                       ALU.add, ALU.mult)
                        hm = work.tile([P, NSLOT], BF16, tag=f"hmy{m}",
                                       name=f"hmy{m}")
                        nc.vector.tensor_add(out=hm, in0=fn, in1=fzd)
                        hmy.append(hm)
                    # dp for my rows + transpose to row-major
                    pdprm = ps1.tile([NSLOT, H2], F32, tag="pdprm",
                                     name="pdprm")
                    for m in range(8):
                        pdp = ps1.tile([P, NSLOT], F32, tag="pp8", bufs=2,
                                       name="pdp")
                        mm_acc(pdp, [(w1(("ldf", m, k)), hmy[k])
                                     for k in range(8)])
                        dmy = work.tile([P, NSLOT], F32, tag="dmy",
                                        name="dmy")
                        nc.scalar.activation(dmy, pdp, AF.Identity,
                                             bias=bcol(f"fld{m}"))
                        nc.tensor.transpose(pdprm[:, m * P : (m + 1) * P],
                                            dmy, ident)
                    dprm = work.tile([NSLOT, H2], BF16, name="dprm")
                    nc.scalar.copy(dprm, pdprm)
                    nc.sync.dma_start(out=dp_dram, in_=dprm)

                # prenet replicated on every core (no collectives)
                pre1f = []
                for m in range(8):
                    p1 = ps1.tile([P, B], F32, tag="pg", bufs=2, name="p1")
                    mm_acc(p1, [(w1(("pre1", m)), xdec_sb)])
                    t = const.tile([P, B], BF16, tag=f"pre1f{m}",
                                   name=f"pre1f{m}")
                    nc.scalar.activation(t, p1, AF.Relu,
                                         bias=bcol(f"pre1m{m}"))
                    pre1f.append(t)
                pre2f = []
                for m in range(4):
                    p2 = ps1.tile([P, B], F32, tag="pg", bufs=2, name="p2")
                    mm_acc(p2, [(w1(("pre2", m, k)), pre1f[k])
                                for k in range(8)])
                    t = const.tile([P, B], BF16, tag=f"pre2f{m}",
                                   name=f"pre2f{m}")
                    nc.scalar.activation(t, p2, AF.Relu,
                                         bias=bcol(f"pre2m{m}"))
                    pre2f.append(t)

                # sharded GRU-att (h_att output + phase-3 AllGather)
                rz = ps1.tile([P, 2 * B], F32, tag="rz", name="rz")
                for g in (0, 1):
                    reg = rz[:, g * B : (g + 1) * B]
                    pairs = [(w1(("att_ih", g, k)), pre2f[k]) for k in range(4)]
                    if not zeros_h:
                        pairs += [(w1(("att_hh", g, k)), h0att_t[k])
                                  for k in range(8)]
                    mm_acc(reg, pairs)
                gin = ps1.tile([P, B], F32, tag="gin", name="gin")
                mm_acc(gin, [(w1(("att_ih", 2, k)), pre2f[k]) for k in range(4)])
                if zeros_h:
                    ghn = None
                else:
                    ghn = ps1.tile([P, B], F32, tag="ghn", name="ghn")
                    mm_acc(ghn, [(w1(("att_hh", 2, k)), h0att_t[k])
                                 for k in range(8)])
                hatt_c = gru_combine(rz, gin, ghn, "att", hhalf_sb[:, 0:B])
                nc.sync.dma_start(out=o_hatt[:], in_=hatt_c)

                if not zeros_h:
                    # dec_proj via K-sharded GEMM + ReduceScatter
                    hatt_bf = work.tile([P, B], BF16, name="hatt_bf")
                    nc.vector.tensor_copy(out=hatt_bf, in_=hatt_c)
                    pdpt = ps1.tile([B, H2], F32, tag="pdpt", name="pdpt")
                    for h in range(2):
                        nc.tensor.matmul(pdpt[:, h * 512 : (h + 1) * 512],
                                         hatt_bf, w1(("ldT", h)),
                                         start=True, stop=True)
                    dpt_sb = work.tile([B, H2], F32, name="dpt_sb")
                    nc.scalar.copy(dpt_sb, pdpt)
                    rs_in = dram.tile([B, H2], F32, name="rs_in")
                    nc.sync.dma_start(out=rs_in, in_=dpt_sb)
                    rs_out = dram.tile([NSLOT, H2], F32, name="rs_out")
                    nc.gpsimd.collective_compute(
                        "ReduceScatter", ALU.add, replica_groups=rg,
                        ins=[rs_in.opt()], outs=[rs_out.opt()])
                    dprm_f = work.tile([NSLOT, H2], F32, name="dprm_f")
                    nc.sync.dma_start(out=dprm_f, in_=rs_out[:])
                    bld_sb = work.tile([NSLOT, H2], F32, name="bld_sb")
                    nc.sync.dma_start(out=bld_sb, in_=d_bldrm[:])
                    dprm2 = work.tile([NSLOT, H2], BF16, name="dprm2")
                    nc.vector.tensor_add(out=dprm2, in0=dprm_f, in1=bld_sb)
                    nc.sync.dma_start(out=dp_dram, in_=dprm2)

            # NOTE: the h_att AllGather is deliberately emitted AFTER the
            # attention loop — collective triggers live on the in-order
            # GpSimd queue, and an earlier emission blocks the dps
            # broadcast DMAs behind the collective.

            # ---------------- phase 2: attention ----------------
            ag_at_in = dram.tile([NSLOT, H2], F32, name="ag_at_in")
            ag_at = dram.tile([N, H2], F32, name="ag_at")
            with tc.tile_pool(name="ps2", bufs=1, space="PSUM") as ps2:
                for s in range(NSLOT if stage >= 2 else 0):
                    # broadcast dp row s across 128 partitions
                    # dps holds dp row s broadcast over partitions, twice
                    # along free so chunk PAIRS share one add/tanh
                    dps = work.tile([P, 2 * H2], BF16, tag="dps", bufs=3,
                                    name="dps")
                    row = dp_dram[s : s + 1, :]
                    bc = bass.AP(tensor=row.tensor, offset=row.offset,
                                 ap=[[0, P], [0, 2], row.ap[-1]])
                    nc.gpsimd.dma_start(
                        out=dps[:].rearrange("p (two h) -> p two h", two=2),
                        in_=bc)

                    score = work.tile([P, 8], F32, tag="score", name="score")
                    # chunk pairs: [128, 2048] tiles, one add + one tanh per
                    # pair; score reductions stay per-chunk and are emitted
                    # after so DVE never stalls behind ACT's tanh.
                    halves = []
                    npair = (Ts[s] + 1) // 2
                    for pp in range(npair):
                        c0 = 2 * pp
                        nch = min(2, Ts[s] - c0)
                        at = work.tile([P, 2 * H2], BF16, tag="at", bufs=12,
                                       name="at")
                        for j in range(nch):
                            base = (int(chunk_base[s]) + c0 + j) * P
                            nc.sync.dma_start(
                                out=at[:, j * H2 : (j + 1) * H2],
                                in_=d_attw[base : base + P, :])
                        w = nch * H2
                        nc.vector.tensor_add(out=at[:, 0:w], in0=at[:, 0:w],
                                             in1=dps[:, 0:w])
                        nc.scalar.activation(at[:, 0:w], at[:, 0:w], AF.Tanh)
                        for j in range(nch):
                            halves.append((c0 + j,
                                           at[:, j * H2 : (j + 1) * H2]))
                    for cc, ath in halves:
                        nc.vector.scalar_tensor_tensor(
                            ath, ath, 0.0, wb_sb, ALU.add, ALU.mult,
                            accum_out=score[:, cc : cc + 1])

                    # exp, mask, row-sums
                    escore = work.tile([P, 8], F32, tag="escore", name="escore")
                    nc.scalar.activation(escore[:, 0 : Ts[s]],
                                         score[:, 0 : Ts[s]], AF.Exp,
                                         bias=bcol("battn"))
                    wt = work.tile([P, 8], BF16, tag="wt", name="wt")
                    psums = work.tile([P, 1], F32, tag="psums", name="psums")
                    nc.vector.scalar_tensor_tensor(
                        wt[:, 0 : Ts[s]], escore[:, 0 : Ts[s]], 0.0,
                        mask_sb[:, s * 8 : s * 8 + Ts[s]], ALU.add, ALU.mult,
                        accum_out=psums)
                    # denom = max(sum, 1e-12); rec = 1/denom
                    pd = ps2.tile([1, 1], F32, tag="pd", name="pd")
                    nc.tensor.matmul(pd, bcol("ones"), psums, start=True,
                                     stop=True)
                    dsb = work.tile([1, 1], F32, tag="dsb", name="dsb")
                    nc.scalar.copy(dsb, pd)
                    nc.vector.tensor_scalar_max(dsb, dsb, 1e-12)
                    rec = work.tile([1, 1], F32, tag="rec", name="rec")
                    nc.vector.reciprocal(rec, dsb)

                    # weighted sum over enc chunks
                    at0 = ps2.tile([1, 512], F32, tag="at0", bufs=2, name="at0")
                    at1 = ps2.tile([1, 512], F32, tag="at1", bufs=2, name="at1")
                    for cc in range(Ts[s]):
                        base = (int(chunk_base[s]) + cc) * P
                        ec = work.tile([P, H2], BF16, tag="ec", bufs=10,
                                       name="ec")
                        nc.sync.dma_start(out=ec,
                                          in_=d_enc[base : base + P, :])
                        st, sp = cc == 0, cc == Ts[s] - 1
                        nc.tensor.matmul(at0, wt[:, cc : cc + 1], ec[:, 0:512],
                                         start=st, stop=sp)
                        nc.tensor.matmul(at1, wt[:, cc : cc + 1], ec[:, 512:H2],
                                         start=st, stop=sp)
                    attn_s = work.tile([1, H2], F32, tag="attn_s", bufs=2,
                                       name="attn_s")
                    nc.scalar.activation(attn_s[:, 0:512], at0,
                                         AF.Copy, scale=rec[:, 0:1])
                    nc.scalar.activation(attn_s[:, 512:H2], at1,
                                         AF.Copy, scale=rec[:, 0:1])
                    nc.sync.dma_start(out=ag_at_in[s : s + 1, :], in_=attn_s)

            # h_att AllGather (phase-3 input), then attention AllGather
            ag_h = allgather(hatt_c, H2, "hatt")
            hattf = load_k_tiles(ag_h, 8, "hattf", cast=True)
            if stage >= 2:
                nc.gpsimd.collective_compute(
                    "AllGather", ALU.bypass, replica_groups=rg,
                    ins=[ag_at_in.opt()], outs=[ag_at.opt()])

            # ---------------- phase 3 ----------------
            nc.sync.dma_start(out=wts3_sb, in_=d_wts3[:])
            with tc.tile_pool(name="ps3", bufs=1, space="PSUM") as ps3:
              if stage >= 3:
                # feature-major attn tiles via PE transpose
                attn_nat = work.tile([N, H2], F32, name="attn_nat")
                nc.sync.dma_start(out=attn_nat, in_=ag_at[:])
                attnf = []
                for k in range(8):
                    ptp = ps3.tile([P, B], F32, tag="ptp", bufs=2, name="ptp")
                    nc.tensor.transpose(ptp, attn_nat[:, k * P : (k + 1) * P],
                                        ident[0:N, 0:N])
                    tkf = const.tile([P, B], BF16, tag=f"attnf{k}",
                                     name=f"attnf{k}")
                    nc.scalar.copy(tkf, ptp)
                    attnf.append(tkf)
                decin = attnf + hattf  # 16 K-tiles of [128, 64]

                psc = ps3.tile([P, B], F32, tag="psc", name="psc")
                mm_acc(psc, [(w3(("sc", k)), decin[k])
                             for k in list(range(8, 16)) + list(range(8))])

                rz1 = ps3.tile([P, 2 * B], F32, tag="rz", name="rz1")
                for g in (0, 1):
                    reg = rz1[:, g * B : (g + 1) * B]
                    pairs = [(w3(("d1_ih", g, k)), decin[k])
                             for k in list(range(8, 16)) + list(range(8))]
                    if not zeros_h:
                        pairs += [(w3(("d1_hh", g, k)), h0d1_t[k])
                                  for k in range(8)]
                    mm_acc(reg, pairs)
                gin1 = ps3.tile([P, B], F32, tag="gin", name="gin1")
                mm_acc(gin1, [(w3(("d1_ih", 2, k)), decin[k])
                              for k in list(range(8, 16)) + list(range(8))])
                if zeros_h:
                    ghn1 = None
                else:
                    ghn1 = ps3.tile([P, B], F32, tag="ghn", name="ghn1")
                    mm_acc(ghn1, [(w3(("d1_hh", 2, k)), h0d1_t[k])
                                  for k in range(8)])
                hd1_c = gru_combine(rz1, gin1, ghn1, "d1",
                                    hhalf_sb[:, B : 2 * B])
                nc.sync.dma_start(out=o_hd1[:], in_=hd1_c)

                r2_c = work.tile([P, B], F32, name="r2_c")
                nc.vector.tensor_add(out=r2_c, in0=psc, in1=hd1_c)
                ag_r2 = allgather(r2_c, H2, "r2")
                r2f = load_k_tiles(ag_r2, 8, "r2f", cast=True)

                rz2 = ps3.tile([P, 2 * B], F32, tag="rz", name="rz2")
                for g in (0, 1):
                    reg = rz2[:, g * B : (g + 1) * B]
                    pairs = [(w3(("d2_ih", g, k)), r2f[k]) for k in range(8)]
                    if not zeros_h:
                        pairs += [(w3(("d2_hh", g, k)), h0d2_t[k])
                                  for k in range(8)]
                    mm_acc(reg, pairs)
                gin2 = ps3.tile([P, B], F32, tag="gin", name="gin2")
                mm_acc(gin2, [(w3(("d2_ih", 2, k)), r2f[k]) for k in range(8)])
                if zeros_h:
                    ghn2 = None
                else:
                    ghn2 = ps3.tile([P, B], F32, tag="ghn", name="ghn2")
                    mm_acc(ghn2, [(w3(("d2_hh", 2, k)), h0d2_t[k])
                                  for k in range(8)])
                hd2_c = gru_combine(rz2, gin2, ghn2, "d2",
                                    hhalf_sb[:, 2 * B : 3 * B])
                nc.sync.dma_start(out=o_hd2[:], in_=hd2_c)

                r3_c = work.tile([P, B], F32, name="r3_c")
                nc.vector.tensor_add(out=r3_c, in0=r2_c, in1=hd2_c)

                # output GEMM, K-sharded; each core emits its PARTIAL
                # product (bias pre-divided by 8) and the host sums the 8
                # partials — no device AllReduce on the tail.
                r3bf = work.tile([P, B], BF16, name="r3bf")
                nc.vector.tensor_copy(out=r3bf, in_=r3_c)
                for m in range(2):
                    pout = ps3.tile([80, B], F32, tag="pout", name="pout")
                    nc.tensor.matmul(pout, w3(("outT", m)), r3bf,
                                     start=True, stop=True)
                    osb = work.tile([80, B], F32, tag="osb", name="osb")
                    nc.scalar.activation(osb, pout, AF.Identity,
                                         bias=bcol(f"bout{m}")[0:80])
                    nc.sync.dma_start(out=o_out[m * 80 : (m + 1) * 80, :],
                                      in_=osb)

    nc.compile()
    return nc


# ---------------------------------------------------------------------------
# entry point
# ---------------------------------------------------------------------------
_PROGRAM_CACHE = {}


def _get_program(Ts, zeros_h):
    key = (tuple(Ts), zeros_h)
    if key not in _PROGRAM_CACHE:
        _PROGRAM_CACHE[key] = build_program(Ts, zeros_h)
    return _PROGRAM_CACHE[key]


def run_device(inputs, trace=False, **kw):
    in_maps, perm, Ts, zeros_h = _host_prep(inputs)
    nc = _get_program(Ts, zeros_h)
    res = run_bass_kernel_spmd(nc, in_maps, list(range(NCORE)), trace=trace,
                               **kw)
    return res, perm


def _assemble(results, perm):
    inv = np.empty(N, np.int64)
    inv[perm] = np.arange(N)

    def gather(name, rows):
        full = np.concatenate([results[c][name][:rows] for c in range(NCORE)],
                              axis=0)
        return np.ascontiguousarray(full.T[inv])

    h_att = gather("o_hatt", P)
    h_d1 = gather("o_hd1", P)
    h_d2 = gather("o_hd2", P)
    out_full = np.sum([results[c]["o_out"].astype(np.float64)
                       for c in range(NCORE)], axis=0).astype(np.float32)
    out = np.ascontiguousarray(out_full.T[inv]).reshape(N, R, O)
    return out, h_att, h_d1, h_d2


def kernel(**inputs):
    res, perm = run_device(inputs)
    return _assemble(res.results, perm)
